# revision 14
# baseline (speedup 1.0000x reference)
"""PhaseSimilarityTransformerLayer on 8 TRN2 NeuronCores.

Sharding: tensor-parallel over heads. 16 heads / 8 cores = 2 heads/core,
i.e. each core owns a 128-wide slice of d_model for q/k/v/attn_out.

Per-core device program (SPMD — identical program, per-core weight slices):
  phase0: L2-normalize phase tags, PE-transpose -> pnT [64, T]
  phase1: q/k/v projections from d-major content (ctT), streamed per
          512-token block.  q/k kept o-major [128, T]; v token-major.
  phase2: per (b): phase-bias pb_b = pnT_b.T @ pnT_b (symmetric, so the
          [k,q] orientation equals [q,k]); per (b,h,q-tile): logits
          (contract hd=64) + pb -> exp (no max subtraction: logits are
          bounded by construction) -> normalize -> attn_weights out,
          PE-transpose P tiles -> AV matmul -> attn_outT [o-slice, T]
          (+bv folded into AV eviction since rows of P sum to 1).
  phase3: partial out-projection outp = attn_outT.T @ WoT_slice  [T, D]
Host: out = sum_c outp_c + bo;  attn_weights = concat_c over head axis.
"""

import os
import numpy as np
import ml_dtypes

B, K, DM, H, HD, PH = 4, 1024, 1024, 16, 64, 64
T = B * K  # 4096
P = 128
NCORES = 8
HPC = H // NCORES  # heads per core

# "f32" (safe) or "bf16" (fast matmuls, fp32 softmax/psum)
PRECISION = os.environ.get("KPREC", "f32")


def build_nc(phase_scale: float, precision: str = PRECISION, repeats: int = 1):
    import concourse.bass as bass
    import concourse.bacc as bacc
    import concourse.mybir as mybir
    import concourse.tile as tile
    from concourse.masks import make_identity

    dt = mybir.dt
    AF = mybir.ActivationFunctionType
    f32 = dt.float32
    mmdt = f32 if precision == "f32" else dt.bfloat16

    nc = bacc.Bacc("TRN2", target_bir_lowering=False, debug=False,
                   num_devices=NCORES)

    # ---- DRAM I/O ----
    ctT = nc.dram_tensor("ctT", [P, 8, T], mmdt, kind="ExternalInput").ap()
    phs = nc.dram_tensor("phs", [P, T // P, PH], f32, kind="ExternalInput").ap()
    wq = nc.dram_tensor("wq", [P, 8, P], mmdt, kind="ExternalInput").ap()
    wk = nc.dram_tensor("wk", [P, 8, P], mmdt, kind="ExternalInput").ap()
    wv = nc.dram_tensor("wv", [P, 8, P], mmdt, kind="ExternalInput").ap()
    wo = nc.dram_tensor("wo", [P, 2, 512], mmdt, kind="ExternalInput").ap()
    bqs = nc.dram_tensor("bqs", [P, 1], f32, kind="ExternalInput").ap()
    bk_ = nc.dram_tensor("bk", [P, 1], f32, kind="ExternalInput").ap()
    bv_ = nc.dram_tensor("bv", [P, 1], f32, kind="ExternalInput").ap()
    attnw = nc.dram_tensor("attnw", [B, HPC, K, K], f32,
                           kind="ExternalOutput").ap()
    outp = nc.dram_tensor("outp", [T, DM], f32, kind="ExternalOutput").ap()

    with tile.TileContext(nc) as tc:
      for _rep in range(repeats):
        with tc.tile_pool(name="persist", bufs=1) as persist:
            ident = persist.tile([P, P], f32)
            make_identity(nc, ident)

            wq_sb = persist.tile([P, 8, P], mmdt)
            wk_sb = persist.tile([P, 8, P], mmdt)
            wv_sb = persist.tile([P, 8, P], mmdt)
            wo_sb = persist.tile([P, 2, 512], mmdt)
            bq_sb = persist.tile([P, 1], f32)
            bk_sb = persist.tile([P, 1], f32)
            bv_sb = persist.tile([P, 1], f32)
            for dst, src in ((wq_sb, wq), (wk_sb, wk), (wv_sb, wv),
                             (wo_sb, wo), (bq_sb, bqs), (bk_sb, bk_),
                             (bv_sb, bv_)):
                nc.sync.dma_start(dst[:], src)

            pnT = persist.tile([PH, T], f32)
            pnTs = persist.tile([PH, T], f32)  # scaled by phase_scale
            # head-major [hd, h, t]: PE operands stay at partition offset 0
            q_sb = persist.tile([HD, HPC, T], mmdt)
            k_sb = persist.tile([HD, HPC, T], mmdt)
            v_sb = persist.tile([P, T // P, P], mmdt)
            ao_sb = persist.tile([P, T], mmdt)

            # ---------------- phase 0: phase normalization ----------------
            with tc.tile_pool(name="ph0", bufs=2) as ph0, \
                 tc.tile_pool(name="ph0ps", bufs=2, space="PSUM") as ph0ps:
                for tt in range(T // P):
                    ph_t = ph0.tile([P, PH], f32, tag="ph_t")
                    nc.sync.dma_start(ph_t[:], phs[:, tt, :])
                    scr = ph0.tile([P, PH], f32, tag="scr")
                    sq = ph0.tile([P, 1], f32, tag="sq")
                    nc.scalar.activation(scr, ph_t, AF.Square, accum_out=sq)
                    nrm = ph0.tile([P, 1], f32, tag="nrm")
                    nc.scalar.sqrt(nrm, sq)
                    nc.vector.tensor_scalar_max(nrm, nrm, 1e-12)
                    rn = ph0.tile([P, 1], f32, tag="rn")
                    nc.vector.reciprocal(rn, nrm)
                    nc.vector.tensor_scalar_mul(scr, ph_t, rn)
                    pnps = ph0ps.tile([PH, P], f32, tag="pnps")
                    nc.tensor.transpose(pnps, scr, ident)
                    nc.scalar.copy(pnT[:, tt * P:(tt + 1) * P], pnps)
                    nc.scalar.activation(pnTs[:, tt * P:(tt + 1) * P], pnps,
                                         AF.Copy, scale=float(phase_scale))

            # ---------------- phase 1: q/k/v projections ------------------
            with tc.tile_pool(name="ph1", bufs=2) as ph1, \
                 tc.tile_pool(name="ph1ps", bufs=2, space="PSUM") as ph1ps:
                for tb in range(8):  # 512-token blocks
                    ct_t = ph1.tile([P, 8, 512], mmdt, tag="ct")
                    nc.sync.dma_start(ct_t[:], ctT[:, :, tb * 512:(tb + 1) * 512])
                    q_ps = ph1ps.tile([P, 512], f32, tag="q")
                    k_ps = ph1ps.tile([P, 512], f32, tag="k")
                    for dc in range(8):
                        nc.tensor.matmul(q_ps, wq_sb[:, dc, :], ct_t[:, dc, :],
                                         start=(dc == 0), stop=(dc == 7))
                    for dc in range(8):
                        nc.tensor.matmul(k_ps, wk_sb[:, dc, :], ct_t[:, dc, :],
                                         start=(dc == 0), stop=(dc == 7))
                    ts512 = slice(tb * 512, (tb + 1) * 512)
                    for h in range(HPC):
                        hsl = slice(h * HD, (h + 1) * HD)
                        nc.scalar.activation(q_sb[:, h, ts512], q_ps[hsl, :],
                                             AF.Identity, bias=bq_sb[hsl, :],
                                             scale=1.0 / np.sqrt(HD))
                        nc.scalar.activation(k_sb[:, h, ts512], k_ps[hsl, :],
                                             AF.Identity, bias=bk_sb[hsl, :])
                    for vt in range(4):
                        v_ps = ph1ps.tile([P, P], f32, tag="v")
                        for dc in range(8):
                            nc.tensor.matmul(
                                v_ps,
                                ct_t[:, dc, vt * P:(vt + 1) * P],
                                wv_sb[:, dc, :],
                                start=(dc == 0), stop=(dc == 7))
                        nc.vector.tensor_copy(v_sb[:, tb * 4 + vt, :], v_ps)

            # ------------- phase 2+3: attention + out-projection ----------
            with tc.tile_pool(name="p2", bufs=3) as p2, \
                 tc.tile_pool(name="ptp", bufs=2) as ptp, \
                 tc.tile_pool(name="outp_sb", bufs=2) as outsb, \
                 tc.tile_pool(name="ps_lg", bufs=2, space="PSUM") as ps_lg, \
                 tc.tile_pool(name="ps_pt", bufs=2, space="PSUM") as ps_pt, \
                 tc.tile_pool(name="ps_av", bufs=2, space="PSUM") as ps_av:
                for b in range(B):
                    bcol = b * K
                    for h in range(HPC):
                        hsl = slice(h * HD, (h + 1) * HD)
                        for qb in range(2):  # 512-query blocks
                            pt_sb = ptp.tile([P, 8, 512], mmdt, tag="pt")
                            for qi in range(4):
                                qt = qb * 4 + qi
                                qc = bcol + qt * P
                                lg_ps = ps_lg.tile([P, 2, 512], f32, tag="lg")
                                for kb in range(2):
                                    ksl = slice(bcol + kb * 512,
                                                bcol + (kb + 1) * 512)
                                    nc.tensor.matmul(
                                        lg_ps[:, kb, :],
                                        pnTs[:, qc:qc + P],
                                        pnT[:, ksl],
                                        start=True, stop=False)
                                    nc.tensor.matmul(
                                        lg_ps[:, kb, :],
                                        q_sb[:, h, qc:qc + P],
                                        k_sb[:, h, ksl],
                                        start=False, stop=True)
                                pt_t = p2.tile([P, K], f32, tag="Pt")
                                ssum = p2.tile([P, 1], f32, tag="ssum")
                                nc.scalar.activation(pt_t, lg_ps[:], AF.Exp,
                                                     accum_out=ssum)
                                rs = p2.tile([P, 1], f32, tag="rs")
                                nc.vector.reciprocal(rs, ssum)
                                nc.vector.tensor_scalar_mul(pt_t, pt_t, rs)
                                nc.sync.dma_start(
                                    attnw[b, h, qt * P:(qt + 1) * P, :], pt_t)
                                for kh in range(2):
                                    tp_ps = ps_pt.tile([P, 4, P], f32, tag="tp")
                                    for kj in range(4):
                                        kc = kh * 4 + kj
                                        nc.tensor.transpose(
                                            tp_ps[:, kj, :],
                                            pt_t[:, kc * P:(kc + 1) * P],
                                            ident)
                                    nc.scalar.copy(
                                        pt_sb[:, kh * 4:(kh + 1) * 4,
                                              qi * P:(qi + 1) * P],
                                        tp_ps)
                            av_ps = ps_av.tile([HD, 512], f32, tag="av")
                            for kc in range(8):
                                nc.tensor.matmul(
                                    av_ps,
                                    v_sb[:, b * 8 + kc, hsl],
                                    pt_sb[:, kc, :],
                                    start=(kc == 0), stop=(kc == 7))
                            nc.scalar.activation(
                                ao_sb[hsl, bcol + qb * 512:
                                      bcol + (qb + 1) * 512],
                                av_ps, AF.Identity, bias=bv_sb[hsl, :])
                    # out-projection for batch b's tokens (partial over o)
                    for ti in range(8):
                        tcol = bcol + ti * P
                        for db in range(2):
                            op_ps = ps_av.tile([P, 512], f32, tag="av")
                            nc.tensor.matmul(op_ps, ao_sb[:, tcol:tcol + P],
                                             wo_sb[:, db, :],
                                             start=True, stop=True)
                            ot = outsb.tile([P, 512], f32, tag="ot")
                            nc.vector.tensor_copy(ot, op_ps)
                            nc.sync.dma_start(
                                outp[tcol:tcol + P, db * 512:(db + 1) * 512],
                                ot)
    nc.compile()
    return nc


def make_in_maps(inputs, precision: str = PRECISION):
    mmnp = np.float32 if precision == "f32" else ml_dtypes.bfloat16
    ct = np.asarray(inputs["content"], np.float32).reshape(T, DM)
    ctT = np.ascontiguousarray(
        ct.T.reshape(8, P, T).transpose(1, 0, 2)).astype(mmnp)
    phs = np.ascontiguousarray(
        np.asarray(inputs["phase"], np.float32).reshape(T // P, P, PH)
        .transpose(1, 0, 2))

    base = {"ctT": ctT, "phs": phs}
    in_maps = []
    for c in range(NCORES):
        osl = slice(c * P, (c + 1) * P)
        m = dict(base)
        for nm in ("wq", "wk", "wv"):
            W = np.asarray(inputs["W" + nm[1]], np.float32)
            WTs = W[osl, :].T  # [D, 128]
            m[nm] = np.ascontiguousarray(
                WTs.reshape(8, P, P).transpose(1, 0, 2)).astype(mmnp)
        Wo = np.asarray(inputs["Wo"], np.float32)
        m["wo"] = np.ascontiguousarray(
            Wo[:, osl].T.reshape(P, 2, 512)).astype(mmnp)
        m["bqs"] = (np.asarray(inputs["bq"], np.float32)[osl] /
                    np.sqrt(HD)).reshape(P, 1).astype(np.float32)
        m["bk"] = np.asarray(inputs["bk"], np.float32)[osl].reshape(P, 1)
        m["bv"] = np.asarray(inputs["bv"], np.float32)[osl].reshape(P, 1)
        in_maps.append(m)
    return in_maps


def gather_outputs(results, inputs):
    out = np.zeros((T, DM), np.float32)
    for r in results:
        out += r["outp"]
    out += np.asarray(inputs["bo"], np.float32)[None, :]
    attn = np.concatenate([r["attnw"] for r in results], axis=1)
    return out.reshape(B, K, DM), attn


def _run(inputs, trace=False):
    from concourse.bass_utils import run_bass_kernel_spmd

    nc = build_nc(float(np.asarray(inputs["phase_scale"])))
    in_maps = make_in_maps(inputs)
    kw = {}
    if trace:
        kw = dict(trace=True, trace_cores=[0])
    res = run_bass_kernel_spmd(nc, in_maps, core_ids=list(range(NCORES)), **kw)
    return gather_outputs(res.results, inputs), res


def kernel(**inputs):
    return _run(inputs)[0]


# revision 15
# speedup vs baseline: 1.2781x; 1.2781x over previous
"""PhaseSimilarityTransformerLayer on 8 TRN2 NeuronCores.

Sharding: tensor-parallel over heads. 16 heads / 8 cores = 2 heads/core,
i.e. each core owns a 128-wide slice of d_model for q/k/v/attn_out.

Per-core device program (SPMD — identical program, per-core weight slices):
  phase0: L2-normalize phase tags, PE-transpose -> pnT [64, T]
  phase1: q/k/v projections from d-major content (ctT), streamed per
          512-token block.  q/k kept o-major [128, T]; v token-major.
  phase2: per (b): phase-bias pb_b = pnT_b.T @ pnT_b (symmetric, so the
          [k,q] orientation equals [q,k]); per (b,h,q-tile): logits
          (contract hd=64) + pb -> exp (no max subtraction: logits are
          bounded by construction) -> normalize -> attn_weights out,
          PE-transpose P tiles -> AV matmul -> attn_outT [o-slice, T]
          (+bv folded into AV eviction since rows of P sum to 1).
  phase3: partial out-projection outp = attn_outT.T @ WoT_slice  [T, D]
Host: out = sum_c outp_c + bo;  attn_weights = concat_c over head axis.
"""

import os
import numpy as np
import ml_dtypes

B, K, DM, H, HD, PH = 4, 1024, 1024, 16, 64, 64
T = B * K  # 4096
P = 128
NCORES = 8
HPC = H // NCORES  # heads per core

# "f32" (safe) or "bf16" (fast matmuls, fp32 softmax/psum)
PRECISION = os.environ.get("KPREC", "f32")


def build_nc(phase_scale: float, precision: str = PRECISION, repeats: int = 1):
    import concourse.bass as bass
    import concourse.bacc as bacc
    import concourse.mybir as mybir
    import concourse.tile as tile
    from concourse.masks import make_identity

    dt = mybir.dt
    AF = mybir.ActivationFunctionType
    f32 = dt.float32
    mmdt = f32 if precision == "f32" else dt.bfloat16

    nc = bacc.Bacc("TRN2", target_bir_lowering=False, debug=False,
                   num_devices=NCORES)

    # ---- DRAM I/O ----
    ctT = nc.dram_tensor("ctT", [P, 8, T], mmdt, kind="ExternalInput").ap()
    phs = nc.dram_tensor("phs", [P, T // P, PH], f32, kind="ExternalInput").ap()
    wq = nc.dram_tensor("wq", [P, 8, P], mmdt, kind="ExternalInput").ap()
    wk = nc.dram_tensor("wk", [P, 8, P], mmdt, kind="ExternalInput").ap()
    wv = nc.dram_tensor("wv", [P, 8, P], mmdt, kind="ExternalInput").ap()
    wo = nc.dram_tensor("wo", [P, 2, 512], mmdt, kind="ExternalInput").ap()
    bqs = nc.dram_tensor("bqs", [P, 1], f32, kind="ExternalInput").ap()
    bk_ = nc.dram_tensor("bk", [P, 1], f32, kind="ExternalInput").ap()
    bv_ = nc.dram_tensor("bv", [P, 1], f32, kind="ExternalInput").ap()
    attnw = nc.dram_tensor("attnw", [B, HPC, K, K], f32,
                           kind="ExternalOutput").ap()
    outp = nc.dram_tensor("outp", [T, DM], f32, kind="ExternalOutput").ap()

    with tile.TileContext(nc) as tc:
      for _rep in range(repeats):
        with tc.tile_pool(name="persist", bufs=1) as persist:
            ident = persist.tile([P, P], f32)
            make_identity(nc, ident)

            wq_sb = persist.tile([P, 8, P], mmdt)
            wk_sb = persist.tile([P, 8, P], mmdt)
            wv_sb = persist.tile([P, 8, P], mmdt)
            wo_sb = persist.tile([P, 2, 512], mmdt)
            bq_sb = persist.tile([P, 1], f32)
            bk_sb = persist.tile([P, 1], f32)
            bv_sb = persist.tile([P, 1], f32)
            for dst, src in ((wq_sb, wq), (wk_sb, wk), (wv_sb, wv),
                             (wo_sb, wo), (bq_sb, bqs), (bk_sb, bk_),
                             (bv_sb, bv_)):
                nc.sync.dma_start(dst[:], src)

            pnT = persist.tile([PH, T], mmdt)
            pnTs = persist.tile([PH, T], mmdt)  # scaled by phase_scale
            # head-major [hd, h, t]: PE operands stay at partition offset 0
            q_sb = persist.tile([HD, HPC, T], mmdt)
            k_sb = persist.tile([HD, HPC, T], mmdt)
            v_sb = persist.tile([P, T // P, P], mmdt)
            ao_sb = persist.tile([P, T], mmdt)

            # ---------------- phase 0: phase normalization ----------------
            with tc.tile_pool(name="ph0", bufs=2) as ph0, \
                 tc.tile_pool(name="ph0ps", bufs=2, space="PSUM") as ph0ps:
                for tt in range(T // P):
                    ph_t = ph0.tile([P, PH], f32, tag="ph_t")
                    nc.sync.dma_start(ph_t[:], phs[:, tt, :])
                    scr = ph0.tile([P, PH], f32, tag="scr")
                    sq = ph0.tile([P, 1], f32, tag="sq")
                    nc.scalar.activation(scr, ph_t, AF.Square, accum_out=sq)
                    nrm = ph0.tile([P, 1], f32, tag="nrm")
                    nc.scalar.sqrt(nrm, sq)
                    nc.vector.tensor_scalar_max(nrm, nrm, 1e-12)
                    rn = ph0.tile([P, 1], f32, tag="rn")
                    nc.vector.reciprocal(rn, nrm)
                    nc.vector.tensor_scalar_mul(scr, ph_t, rn)
                    pnps = ph0ps.tile([PH, P], f32, tag="pnps")
                    nc.tensor.transpose(pnps, scr, ident)
                    nc.scalar.copy(pnT[:, tt * P:(tt + 1) * P], pnps)
                    nc.scalar.activation(pnTs[:, tt * P:(tt + 1) * P], pnps,
                                         AF.Copy, scale=float(phase_scale))

            # ---------------- phase 1: q/k/v projections ------------------
            with tc.tile_pool(name="ph1", bufs=2) as ph1, \
                 tc.tile_pool(name="ph1ps", bufs=2, space="PSUM") as ph1ps:
                for tb in range(8):  # 512-token blocks
                    ct_t = ph1.tile([P, 8, 512], mmdt, tag="ct")
                    nc.sync.dma_start(ct_t[:], ctT[:, :, tb * 512:(tb + 1) * 512])
                    q_ps = ph1ps.tile([P, 512], f32, tag="q")
                    k_ps = ph1ps.tile([P, 512], f32, tag="k")
                    for dc in range(8):
                        nc.tensor.matmul(q_ps, wq_sb[:, dc, :], ct_t[:, dc, :],
                                         start=(dc == 0), stop=(dc == 7))
                    for dc in range(8):
                        nc.tensor.matmul(k_ps, wk_sb[:, dc, :], ct_t[:, dc, :],
                                         start=(dc == 0), stop=(dc == 7))
                    ts512 = slice(tb * 512, (tb + 1) * 512)
                    for h in range(HPC):
                        hsl = slice(h * HD, (h + 1) * HD)
                        nc.scalar.activation(q_sb[:, h, ts512], q_ps[hsl, :],
                                             AF.Identity, bias=bq_sb[hsl, :],
                                             scale=1.0 / np.sqrt(HD))
                        nc.scalar.activation(k_sb[:, h, ts512], k_ps[hsl, :],
                                             AF.Identity, bias=bk_sb[hsl, :])
                    for vt in range(4):
                        v_ps = ph1ps.tile([P, P], f32, tag="v")
                        for dc in range(8):
                            nc.tensor.matmul(
                                v_ps,
                                ct_t[:, dc, vt * P:(vt + 1) * P],
                                wv_sb[:, dc, :],
                                start=(dc == 0), stop=(dc == 7))
                        nc.vector.tensor_copy(v_sb[:, tb * 4 + vt, :], v_ps)

            # ------------- phase 2+3: attention + out-projection ----------
            with tc.tile_pool(name="p2", bufs=3) as p2, \
                 tc.tile_pool(name="ptp", bufs=2) as ptp, \
                 tc.tile_pool(name="outp_sb", bufs=2) as outsb, \
                 tc.tile_pool(name="ps_lg", bufs=2, space="PSUM") as ps_lg, \
                 tc.tile_pool(name="ps_pt", bufs=2, space="PSUM") as ps_pt, \
                 tc.tile_pool(name="ps_av", bufs=2, space="PSUM") as ps_av:
                for b in range(B):
                    bcol = b * K
                    for h in range(HPC):
                        hsl = slice(h * HD, (h + 1) * HD)
                        for qb in range(2):  # 512-query blocks
                            pt_sb = ptp.tile([P, 8, 512], mmdt, tag="pt")
                            for qi in range(4):
                                qt = qb * 4 + qi
                                qc = bcol + qt * P
                                lg_ps = ps_lg.tile([P, 2, 512], f32, tag="lg")
                                for kb in range(2):
                                    ksl = slice(bcol + kb * 512,
                                                bcol + (kb + 1) * 512)
                                    nc.tensor.matmul(
                                        lg_ps[:, kb, :],
                                        pnTs[:, qc:qc + P],
                                        pnT[:, ksl],
                                        start=True, stop=False)
                                    nc.tensor.matmul(
                                        lg_ps[:, kb, :],
                                        q_sb[:, h, qc:qc + P],
                                        k_sb[:, h, ksl],
                                        start=False, stop=True)
                                pt_t = p2.tile([P, K], f32, tag="Pt")
                                ssum = p2.tile([P, 1], f32, tag="ssum")
                                nc.scalar.activation(pt_t, lg_ps[:], AF.Exp,
                                                     accum_out=ssum)
                                rs = p2.tile([P, 1], f32, tag="rs")
                                nc.vector.reciprocal(rs, ssum)
                                nc.vector.tensor_scalar_mul(pt_t, pt_t, rs)
                                nc.sync.dma_start(
                                    attnw[b, h, qt * P:(qt + 1) * P, :], pt_t)
                                for kh in range(2):
                                    tp_ps = ps_pt.tile([P, 4, P], f32, tag="tp")
                                    for kj in range(4):
                                        kc = kh * 4 + kj
                                        nc.tensor.transpose(
                                            tp_ps[:, kj, :],
                                            pt_t[:, kc * P:(kc + 1) * P],
                                            ident)
                                    nc.scalar.copy(
                                        pt_sb[:, kh * 4:(kh + 1) * 4,
                                              qi * P:(qi + 1) * P],
                                        tp_ps)
                            av_ps = ps_av.tile([HD, 512], f32, tag="av")
                            for kc in range(8):
                                nc.tensor.matmul(
                                    av_ps,
                                    v_sb[:, b * 8 + kc, hsl],
                                    pt_sb[:, kc, :],
                                    start=(kc == 0), stop=(kc == 7))
                            nc.scalar.activation(
                                ao_sb[hsl, bcol + qb * 512:
                                      bcol + (qb + 1) * 512],
                                av_ps, AF.Identity, bias=bv_sb[hsl, :])
                    # out-projection for batch b's tokens (partial over o)
                    for ti in range(8):
                        tcol = bcol + ti * P
                        for db in range(2):
                            op_ps = ps_av.tile([P, 512], f32, tag="av")
                            nc.tensor.matmul(op_ps, ao_sb[:, tcol:tcol + P],
                                             wo_sb[:, db, :],
                                             start=True, stop=True)
                            ot = outsb.tile([P, 512], f32, tag="ot")
                            nc.vector.tensor_copy(ot, op_ps)
                            nc.sync.dma_start(
                                outp[tcol:tcol + P, db * 512:(db + 1) * 512],
                                ot)
    nc.compile()
    return nc


def make_in_maps(inputs, precision: str = PRECISION):
    mmnp = np.float32 if precision == "f32" else ml_dtypes.bfloat16
    ct = np.asarray(inputs["content"], np.float32).reshape(T, DM)
    ctT = np.ascontiguousarray(
        ct.T.reshape(8, P, T).transpose(1, 0, 2)).astype(mmnp)
    phs = np.ascontiguousarray(
        np.asarray(inputs["phase"], np.float32).reshape(T // P, P, PH)
        .transpose(1, 0, 2))

    base = {"ctT": ctT, "phs": phs}
    in_maps = []
    for c in range(NCORES):
        osl = slice(c * P, (c + 1) * P)
        m = dict(base)
        for nm in ("wq", "wk", "wv"):
            W = np.asarray(inputs["W" + nm[1]], np.float32)
            WTs = W[osl, :].T  # [D, 128]
            m[nm] = np.ascontiguousarray(
                WTs.reshape(8, P, P).transpose(1, 0, 2)).astype(mmnp)
        Wo = np.asarray(inputs["Wo"], np.float32)
        m["wo"] = np.ascontiguousarray(
            Wo[:, osl].T.reshape(P, 2, 512)).astype(mmnp)
        m["bqs"] = (np.asarray(inputs["bq"], np.float32)[osl] /
                    np.sqrt(HD)).reshape(P, 1).astype(np.float32)
        m["bk"] = np.asarray(inputs["bk"], np.float32)[osl].reshape(P, 1)
        m["bv"] = np.asarray(inputs["bv"], np.float32)[osl].reshape(P, 1)
        in_maps.append(m)
    return in_maps


def gather_outputs(results, inputs):
    out = np.zeros((T, DM), np.float32)
    for r in results:
        out += r["outp"]
    out += np.asarray(inputs["bo"], np.float32)[None, :]
    attn = np.concatenate([r["attnw"] for r in results], axis=1)
    return out.reshape(B, K, DM), attn


def _run(inputs, trace=False):
    from concourse.bass_utils import run_bass_kernel_spmd

    nc = build_nc(float(np.asarray(inputs["phase_scale"])))
    in_maps = make_in_maps(inputs)
    kw = {}
    if trace:
        kw = dict(trace=True, trace_cores=[0])
    res = run_bass_kernel_spmd(nc, in_maps, core_ids=list(range(NCORES)), **kw)
    return gather_outputs(res.results, inputs), res


def kernel(**inputs):
    return _run(inputs)[0]
